# revision 1
# baseline (speedup 1.0000x reference)
"""KingLoss Trainium2 kernel (raw Bass, explicit semaphores).

Masked cross-entropy loss over [N, 10] logits, data-parallel over 8
NeuronCores.  Each core reduces its shard of rows to tiny per-engine
partial-sum tensors on device; the host does the final (cheap) reduction.

Per-row math (epoch % 5 == 0 branch, the one the harness exercises):
    lse_i  = log(sum_c exp(x_ic))
    ce_i   = lse_i - x_{i,t_i}
    p_i    = exp(x_{i,KING} - lse_i)          # softmax prob of class KING
    loss_i = ce_i + (t_i != KING) * p_i
    loss   = mean_i loss_i

Only global sums are needed, so per tile the device accumulates
    Sum lse            (activation Ln with accum_out)
    Sum (t!=K) * p     (fused scalar_tensor_tensor with accum_out)
    Sum (t==c) * x_c   (one fused STT per class c, accum_out)
into per-engine stats rows; the host sums the [128, T*k] partials in f64.

Raw Bass (not Tile): the walrus build in this container accepts at most
one sync-wait per instruction, which Tile's auto-semaphores exceed.  All
waits here are standalone wait_ge instructions, hand-counted:
    act_sem: +1 after each ACT op   (3 per tile: exp, ln, p)
    dve_sem: +1 after reduce, after d-sub, after last class STT
Transitivity makes one dve_sem wait cover both engines for buffer reuse
(DVE's tile-done implies ACT's p which implies ACT's exp, etc.).

Layout: rows spread across 128 partitions via a flat reshape; each
partition holds R consecutive rows (10 contiguous floats per row), so all
DMAs are fully contiguous per partition.
"""

import os
import sys

import numpy as np

for _p in ("/opt/trn_rl_repo", "/root/.axon_site/_ro/trn_rl_repo"):
    if os.path.isdir(_p) and _p not in sys.path:
        sys.path.insert(0, _p)
        break

import concourse.bass as bass
import concourse.mybir as mybir
from concourse.bass_utils import run_bass_kernel_spmd

P = 128          # SBUF partitions
C = 10           # classes
KING = 3
R = 512          # rows per partition per tile
F = R * C        # floats per partition per x tile
N_CORES = 8
NBUF = 3         # pipeline depth (x/e/t/... buffer rotation)

FP32 = mybir.dt.float32
AF = mybir.ActivationFunctionType
OP = mybir.AluOpType
AX = mybir.AxisListType

_BUILT = {}
LAST = {}  # exec_time_ns etc. from the most recent run, for test harnesses


def _build(T, epoch_zero):
    """Build the per-core Bass module. T = tiles per core."""
    NSV = 11 if epoch_zero else 2   # DVE stats slots per tile
    APT = 3 if epoch_zero else 2    # ACT ops per tile
    DPT = 3 if epoch_zero else 2    # dve_sem incs per tile
    nc = bass.Bass()
    x = nc.declare_dram_parameter("x", [T * P, F], FP32, isOutput=False)
    tg = nc.declare_dram_parameter("t", [T * P, R], FP32, isOutput=False)
    out_a = nc.declare_dram_parameter("pa", [P, T], FP32, isOutput=True)
    out_v = nc.declare_dram_parameter("pv", [P, T * NSV], FP32, isOutput=True)

    with (
        nc.sbuf_tensor("xt", [P, NBUF * F], FP32) as xt,
        nc.sbuf_tensor("et", [P, NBUF * F], FP32) as et,
        nc.sbuf_tensor("tt", [P, NBUF * R], FP32) as tt,
        nc.sbuf_tensor("sb", [P, NBUF * R], FP32) as sb,
        nc.sbuf_tensor("lse", [P, NBUF * R], FP32) as lse,
        nc.sbuf_tensor("db", [P, NBUF * R], FP32) as db,
        nc.sbuf_tensor("pb", [P, NBUF * R], FP32) as pb,
        nc.sbuf_tensor("dmy", [P, R], FP32) as dmy,
        nc.sbuf_tensor("sta", [P, T], FP32) as sta,
        nc.sbuf_tensor("stv", [P, T * NSV], FP32) as stv,
        nc.semaphore("dma_x0") as dma_x0,
        nc.semaphore("dma_x1") as dma_x1,
        nc.semaphore("dma_x2") as dma_x2,
        nc.semaphore("dma_t0") as dma_t0,
        nc.semaphore("dma_t1") as dma_t1,
        nc.semaphore("dma_t2") as dma_t2,
        nc.semaphore("act_sem") as act_sem,
        nc.semaphore("dve_sem") as dve_sem,
        nc.semaphore("dma_oa") as dma_oa,
        nc.semaphore("dma_ob") as dma_ob,
        nc.Block() as block,
    ):
        def xtile(b):
            return xt[:, b * F:(b + 1) * F]

        def x3(b):
            return xtile(b).rearrange("p (r c) -> p r c", c=C)

        def etile(b):
            return et[:, b * F:(b + 1) * F]

        def e3(b):
            return etile(b).rearrange("p (r c) -> p r c", c=C)

        def rtile(buf, b):
            return buf[:, b * R:(b + 1) * R]

        dma_x = [dma_x0, dma_x1, dma_x2]
        dma_t = [dma_t0, dma_t1, dma_t2]

        @block.sync
        def _(sync):
            for i in range(T):
                b = i % NBUF
                if i >= NBUF:
                    # DVE tile-done(i-NBUF) transitively covers every
                    # reader (ACT included) of the buffers being reused.
                    sync.wait_ge(dve_sem, DPT * (i - NBUF) + DPT)
                    # order this slot's sem updates (race-detector rule)
                    sync.wait_ge(dma_x[b], 16 * (i // NBUF))
                    sync.wait_ge(dma_t[b], 16 * (i // NBUF))
                sync.dma_start(
                    out=xtile(b), in_=x[i * P:(i + 1) * P, :]
                ).then_inc(dma_x[b], 16)
                sync.dma_start(
                    out=rtile(tt, b), in_=tg[i * P:(i + 1) * P, :]
                ).then_inc(dma_t[b], 16)
            sync.wait_ge(act_sem, APT * T)
            sync.dma_start(out=out_a[:, :], in_=sta[:, :]).then_inc(dma_oa, 16)
            sync.wait_ge(dve_sem, DPT * T)
            sync.dma_start(out=out_v[:, :], in_=stv[:, :]).then_inc(dma_ob, 16)
            sync.wait_ge(dma_oa, 16)
            sync.wait_ge(dma_ob, 16)

        @block.scalar
        def _(scalar):
            for i in range(T):
                b = i % NBUF
                scalar.wait_ge(dma_x[b], 16 * (i // NBUF + 1))
                scalar.activation(etile(b), xtile(b), AF.Exp).then_inc(
                    act_sem, 1)                                   # APT*i+1
                scalar.wait_ge(dve_sem, DPT * i + 1)
                scalar.activation(
                    rtile(lse, b), rtile(sb, b), AF.Ln,
                    accum_out=sta[:, i:i + 1],
                ).then_inc(act_sem, 1)                            # APT*i+2
                if epoch_zero:
                    scalar.wait_ge(dve_sem, DPT * i + 2)
                    scalar.activation(
                        rtile(pb, b), rtile(db, b), AF.Exp
                    ).then_inc(act_sem, 1)                        # APT*i+3

        @block.vector
        def _(vector):
            for i in range(T):
                b = i % NBUF
                col = i * NSV
                vector.wait_ge(act_sem, APT * i + 1)
                vector.tensor_reduce(
                    rtile(sb, b), e3(b), axis=AX.X, op=OP.add
                ).then_inc(dve_sem, 1)                            # DPT*i+1
                vector.wait_ge(act_sem, APT * i + 2)
                if epoch_zero:
                    vector.tensor_tensor(
                        rtile(db, b), x3(b)[:, :, KING], rtile(lse, b),
                        OP.subtract,
                    ).then_inc(dve_sem, 1)                        # DPT*i+2
                    vector.wait_ge(act_sem, APT * i + 3)
                    vector.wait_ge(dma_t[b], 16 * (i // NBUF + 1))
                    vector.scalar_tensor_tensor(
                        dmy[:, :], rtile(tt, b), float(KING), rtile(pb, b),
                        OP.not_equal, OP.mult,
                        accum_out=stv[:, col:col + 1],
                    )
                    ins = []
                    for c in range(C):
                        ins.append(vector.scalar_tensor_tensor(
                            dmy[:, :], rtile(tt, b), float(c), x3(b)[:, :, c],
                            OP.is_equal, OP.mult,
                            accum_out=stv[:, col + 1 + c:col + 2 + c],
                        ))
                    ins[-1].then_inc(dve_sem, 1)                  # DPT*i+3
                else:
                    vector.wait_ge(dma_t[b], 16 * (i // NBUF + 1))
                    vector.scalar_tensor_tensor(
                        dmy[:, :], rtile(tt, b), float(KING), rtile(lse, b),
                        OP.is_equal, OP.mult,
                        accum_out=stv[:, col:col + 1],
                    )
                    vector.scalar_tensor_tensor(
                        dmy[:, :], rtile(tt, b), float(KING), x3(b)[:, :, KING],
                        OP.is_equal, OP.mult,
                        accum_out=stv[:, col + 1:col + 2],
                    ).then_inc(dve_sem, 1)                        # DPT*i+2

    return nc


def kernel(output, target, epoch):
    x = np.ascontiguousarray(np.asarray(output), dtype=np.float32)
    tgt = np.asarray(target)
    epoch_zero = int(epoch) % 5 == 0
    N = x.shape[0]
    n_per = N // N_CORES
    assert N % N_CORES == 0 and n_per % (P * R) == 0
    T = n_per // (P * R)
    tf = tgt.astype(np.float32)

    in_maps = []
    for ci in range(N_CORES):
        in_maps.append({
            "x": x[ci * n_per:(ci + 1) * n_per].reshape(T * P, F),
            "t": tf[ci * n_per:(ci + 1) * n_per].reshape(T * P, R),
        })

    key = (T, epoch_zero)
    if key not in _BUILT:
        _BUILT[key] = _build(T, epoch_zero)
    nc = _BUILT[key]

    trace = bool(os.environ.get("KERNEL_TRACE"))
    res = run_bass_kernel_spmd(nc, in_maps, list(range(N_CORES)), trace=trace)
    LAST["exec_time_ns"] = res.exec_time_ns
    LAST["result"] = res

    NSV = 11 if epoch_zero else 2
    sa = 0.0
    pk = xt_sum = kl = kx = 0.0
    for r in res.results:
        sa += float(r["pa"].astype(np.float64).sum())
        pv = r["pv"].astype(np.float64).reshape(P, T, NSV)
        if epoch_zero:
            pk += float(pv[:, :, 0].sum())
            xt_sum += float(pv[:, :, 1:].sum())
        else:
            kl += float(pv[:, :, 0].sum())
            kx += float(pv[:, :, 1].sum())
    if epoch_zero:
        loss = (sa - xt_sum + pk) / N
    else:
        loss = (kl - kx) / N
    return np.float32(loss)

